# revision 26
# baseline (speedup 1.0000x reference)
"""Trainium2 Bass kernel for MeshMultiHeadHodgeAttentionVertices.

Strategy (8 cores, SPMD single NEFF, per-core data differs only in inputs):
  - Edge rows m sharded 8 ways (MS=1536/core); vertex rows n sharded 8 ways
    (NS=512/core).
  - Neighbor tables are batch-merged: one gather per 128-row tile fetches
    2048-byte rows [K_b0 | K_b1 | X_b0 | X_b1], quartering SWDGE
    descriptor-generation work vs per-(batch, table) gathers.
  - Collectives: AllGather(v_Q, hidden behind phase 2), AllGather(edge
    table, split in two halves to overlap phase 2), AllReduce(x_vert,
    split in two halves to overlap phase 4).
  - v_Q gathers for phase 5 are issued before phase 4 so their SWDGE
    descriptor generation hides behind the phase-4 matmuls.
  - Heavy tensors cast to bf16 on host (matmul inputs + gather tables);
    accumulation in fp32 (PSUM); attention reduces use bf16 add-trees with
    fp32 final steps.
"""

import math
import os

import numpy as np
import ml_dtypes

import concourse.bass as bass
import concourse.bacc as bacc
import concourse.mybir as mybir
import concourse.tile as tile
from concourse.alu_op_type import AluOpType
from concourse.bass_utils import run_bass_kernel_spmd

BF16 = ml_dtypes.bfloat16
F32 = mybir.dt.float32
BF = mybir.dt.bfloat16
I16 = mybir.dt.int16

B, N, M, H, DV = 2, 4096, 12288, 8, 256
DK = DV // H            # 32
KNB = 16                # neighbors
W = 8                   # cores
MS = M // W             # 1536 edge rows per core
NS = N // W             # 512 vertex rows per core
MT = MS // 128          # 12 edge tiles
VT = NS // 128          # 4 vertex tiles
NT = N // 128           # 32 vertex tiles (full)
LN_EPS = 1e-5
ISQ = 1.0 / math.sqrt(DK)
TW = 4 * DV             # edge table row width: [Kb0|Kb1|Xb0|Xb1]
MSH = MS // 2           # edge rows per AG2 half

Exp = mybir.ActivationFunctionType.Exp
Sqrt = mybir.ActivationFunctionType.Sqrt
X = mybir.AxisListType.X

_CACHE: dict = {}


def _build_module(stage=5):
    nc = _emit(stage)
    nc.compile()
    return nc


def _emit(stage):
    nc = bacc.Bacc("TRN2", target_bir_lowering=False, debug=False,
                   num_devices=W)

    # ---- external inputs (per-core shards prepared on host) ----
    xvt_f = nc.dram_tensor("xvt_f", [B, DV, N], BF, kind="ExternalInput")
    xvt_s = nc.dram_tensor("xvt_s", [B, DV, NS], BF, kind="ExternalInput")
    xet_s = nc.dram_tensor("xet_s", [B, DV, MS], BF, kind="ExternalInput")
    w5 = nc.dram_tensor("w5", [5, DV, DV], BF, kind="ExternalInput")
    d0t = nc.dram_tensor("d0t", [B, N, MS], BF, kind="ExternalInput")
    d0n = nc.dram_tensor("d0n", [B, MS, N], BF, kind="ExternalInput")
    gie = nc.dram_tensor("gie", [128, MT * 128], I16, kind="ExternalInput")
    giv = nc.dram_tensor("giv", [128, VT * 128], I16, kind="ExternalInput")
    out = nc.dram_tensor("out", [B, NS, DV], F32, kind="ExternalOutput")

    rg = [list(range(W))]
    nidx = int(os.environ.get("KNIDX", "1024"))
    pair = int(os.environ.get("KPAIR", "1"))

    with tile.TileContext(nc) as tc:
        with (
            tc.tile_pool(name="dram", bufs=1, space="DRAM") as dram,
            tc.tile_pool(name="const", bufs=1) as constp,
            tc.tile_pool(name="resid", bufs=1) as resid,
        ):
            # collective buffers
            ag1_in = dram.tile([NS, 2 * DV], BF, tag="ag1i")
            ag2_in = dram.tile([MS, TW], BF, tag="ag2i")
            ar_in = dram.tile([N, 2 * DV], BF, tag="ari")
            ag1_out = dram.tile([N, 2 * DV], BF, tag="ag1o",
                                addr_space="Shared")
            ag2_out = dram.tile([M, TW], BF, tag="ag2o", addr_space="Shared")
            ara_out = dram.tile([N // 2, 2 * DV], BF, tag="aroa",
                                addr_space="Shared")
            arb_out = dram.tile([N // 2, 2 * DV], BF, tag="arob",
                                addr_space="Shared")
            arl = dram.tile([N, 2 * DV], BF, tag="arl")

            # constants / index tables
            wts = constp.tile([128, 5, 2, DV], BF, tag="wts")
            nc.sync.dma_start(
                out=wts[:], in_=w5[:].rearrange("w (hf p) e -> p w hf e", p=128))
            gie_sb = constp.tile([128, MT, 128], I16, tag="gie")
            nc.sync.dma_start(out=gie_sb[:], in_=gie[:].rearrange(
                "p (t s) -> p t s", t=MT))
            giv_sb = constp.tile([128, VT, 128], I16, tag="giv")
            nc.sync.dma_start(out=giv_sb[:], in_=giv[:].rearrange(
                "p (t s) -> p t s", t=VT))

            eps_t = constp.tile([128, 1], F32, tag="eps")
            nc.vector.memset(eps_t[:], LN_EPS)

            # residents that span multiple phases
            vk = resid.tile([128, B, VT, DV], BF, tag="vk")      # v_K shard
            eq = resid.tile([128, B, MT, DV], BF, tag="eq")      # e_Q shard
            ae = resid.tile([128, MT, B, DV], BF, tag="ae")      # attn edges
            vqs = resid.tile([128, VT, 2 * DV], BF, tag="vqs")   # v_Q staging

            def layernorm(src256, dst256, pool):
                """src256: (128,256) fp32 AP (psum); dst256: (128,256) bf16 AP.
                LN over groups of 32 along free dim."""
                s3 = src256.rearrange("p (h d) -> p h d", d=DK)
                sums = pool.tile([128, H], F32, tag="ln_sums")
                nc.vector.reduce_sum(sums[:], s3, axis=X)
                mean = pool.tile([128, H], F32, tag="ln_mean")
                nc.scalar.mul(mean[:], sums[:], 1.0 / DK)
                cent = pool.tile([128, H, DK], F32, tag="ln_cent")
                nc.vector.tensor_sub(
                    cent[:], s3,
                    mean[:].unsqueeze(2).broadcast_to((128, H, DK)))
                sq = pool.tile([128, H, DK], F32, tag="ln_sq")
                nc.scalar.square(sq[:], cent[:])
                vsum = pool.tile([128, H], F32, tag="ln_vsum")
                nc.vector.reduce_sum(vsum[:], sq[:], axis=X)
                std = pool.tile([128, H], F32, tag="ln_std")
                nc.scalar.activation(std[:], vsum[:], Sqrt,
                                     bias=eps_t[:], scale=1.0 / DK)
                rstd = pool.tile([128, H], F32, tag="ln_rstd")
                nc.vector.reciprocal(rstd[:], std[:])
                nc.vector.tensor_mul(
                    dst256.rearrange("p (h d) -> p h d", d=DK), cent[:],
                    rstd[:].unsqueeze(2).broadcast_to((128, H, DK)))

            # ---------------- phase 1+2: projections, d0 @ v_V ----------
            with (
                tc.tile_pool(name="acts", bufs=1) as actsp,
                tc.tile_pool(name="vvp", bufs=1) as vvp,
                tc.tile_pool(name="proj", bufs=2) as projp,
                tc.tile_pool(name="lnp", bufs=3) as lnp,
                tc.tile_pool(name="ps1", bufs=2, space="PSUM") as ps1,
                tc.tile_pool(name="ps2", bufs=2, space="PSUM") as ps2,
                tc.tile_pool(name="s4p", bufs=3) as s4p,
                tc.tile_pool(name="ps4", bufs=2, space="PSUM") as ps4p,
            ):
                vv = vvp.tile([128, B, NT, DV], BF, tag="vv")    # v_V full
                xvf_sb = actsp.tile([128, B, 2, N], BF, tag="xvf")
                nc.sync.dma_start(
                    out=xvf_sb[:],
                    in_=xvt_f[:].rearrange("b (hf p) n -> p b hf n", p=128))
                xvs_sb = actsp.tile([128, B, 2, NS], BF, tag="xvs")
                nc.sync.dma_start(
                    out=xvs_sb[:],
                    in_=xvt_s[:].rearrange("b (hf p) n -> p b hf n", p=128))
                xes_sb = actsp.tile([128, B, 2, MS], BF, tag="xes")
                nc.sync.dma_start(
                    out=xes_sb[:],
                    in_=xet_s[:].rearrange("b (hf p) n -> p b hf n", p=128))
                # v_Q / v_K first so AG1 launches as early as possible
                for b in range(B):
                    for vt in range(VT):
                        psqk = ps2.tile([128, 2 * DV], F32, tag="psqk")
                        for hf in range(2):
                            nc.tensor.matmul(
                                psqk[:],
                                lhsT=xvs_sb[:, b, hf, vt * 128:(vt + 1) * 128],
                                rhs=wts[:, 0:2, hf, :],
                                start=(hf == 0), stop=(hf == 1))
                        layernorm(psqk[:, 0:DV],
                                  vqs[:, vt, b * DV:(b + 1) * DV], lnp)
                        layernorm(psqk[:, DV:2 * DV], vk[:, b, vt, :], lnp)

                # AG1: v_Q shard -> full v_Q table (hidden behind phase 2)
                nc.sync.dma_start(
                    out=ag1_in[:].rearrange("(vt p) e -> p vt e", p=128),
                    in_=vqs[:])
                nc.gpsimd.collective_compute(
                    "AllGather", AluOpType.bypass, replica_groups=rg,
                    ins=[ag1_in[:].opt()], outs=[ag1_out[:].opt()])

                # e_Q / e_K for edge shard tiles
                for b in range(B):
                    for mt in range(MT):
                        psek = ps2.tile([128, 2 * DV], F32, tag="psqk")
                        for hf in range(2):
                            nc.tensor.matmul(
                                psek[:],
                                lhsT=xes_sb[:, b, hf, mt * 128:(mt + 1) * 128],
                                rhs=wts[:, 3:5, hf, :],
                                start=(hf == 0), stop=(hf == 1))
                        layernorm(psek[:, 0:DV], eq[:, b, mt, :], lnp)
                        ek_t = projp.tile([128, DV], BF, tag="ek_t")
                        layernorm(psek[:, DV:2 * DV], ek_t[:], lnp)
                        nc.sync.dma_start(
                            out=ag2_in[mt * 128:(mt + 1) * 128,
                                       b * DV:(b + 1) * DV],
                            in_=ek_t[:])
                # v_V for all vertex tiles
                for b in range(B):
                    for nt in range(NT):
                        psv = ps1.tile([128, DV], F32, tag="psv")
                        for hf in range(2):
                            nc.tensor.matmul(
                                psv[:],
                                lhsT=xvf_sb[:, b, hf, nt * 128:(nt + 1) * 128],
                                rhs=wts[:, 2, hf, :],
                                start=(hf == 0), stop=(hf == 1))
                        nc.scalar.copy(vv[:, b, nt, :], psv[:])

                if stage < 1:
                    for b in range(B):
                        for vt in range(VT):
                            dbg = projp.tile([128, DV], F32, tag="dbg")
                            nc.scalar.copy(dbg[:], vk[:, b, vt, :])
                            nc.sync.dma_start(
                                out=out[b, vt * 128:(vt + 1) * 128, :],
                                in_=dbg[:])
                    return nc

                # ------------- phase 2: x_edges = d_0 @ v_V -------------
                # mtp-outer so table rows complete in order; AG2 in halves.
                for mtp in range(MT // 2):
                    for b in range(B):
                        dt_t = s4p.tile([128, NT, 256], BF, tag="d0t", bufs=2)
                        nc.sync.dma_start(
                            out=dt_t[:],
                            in_=d0t[b].rearrange("(nt p) m -> p nt m", p=128)
                            [:, :, mtp * 256:(mtp + 1) * 256])
                        for sub in range(2):
                            mt = mtp * 2 + sub
                            ps4 = ps4p.tile([128, DV], F32, tag="ps4")
                            for nt in range(NT):
                                nc.tensor.matmul(
                                    ps4[:],
                                    lhsT=dt_t[:, nt,
                                              sub * 128:(sub + 1) * 128],
                                    rhs=vv[:, b, nt, :],
                                    start=(nt == 0), stop=(nt == NT - 1))
                            xe_t = s4p.tile([128, DV], BF, tag="xe_t")
                            nc.scalar.copy(xe_t[:], ps4[:])
                            nc.sync.dma_start(
                                out=ag2_in[mt * 128:(mt + 1) * 128,
                                           (2 + b) * DV:(3 + b) * DV],
                                in_=xe_t[:])
                # AG2: [e_K | x_edges] shards -> full edge table
                nc.gpsimd.collective_compute(
                    "AllGather", AluOpType.bypass, replica_groups=rg,
                    ins=[ag2_in[:].opt()], outs=[ag2_out[:].opt()])

                if stage < 2:
                    for b in range(B):
                        for vt in range(VT):
                            dbg2 = projp.tile([128, DV], F32, tag="dbg2")
                            nc.scalar.copy(dbg2[:], vk[:, b, vt, :])
                            nc.sync.dma_start(
                                out=out[b, vt * 128:(vt + 1) * 128, :],
                                in_=dbg2[:])
                    return nc

            def gather_tile(pool_or_ap, tag, table_ap, idxs_row, width):
                """Gather (128, KNB, width) bf16 rows using idx columns
                idxs_row (128, 128) = 2048 idxs, j-major.  pool_or_ap is a
                tile pool (allocates) or a destination AP."""
                if hasattr(pool_or_ap, "tile"):
                    alloc = pool_or_ap.tile([128, KNB, width], BF, tag=tag,
                                            name=tag)
                    gt = alloc[:]
                else:
                    gt = pool_or_ap
                    alloc = None
                if nidx == 2048:
                    nc.gpsimd.dma_gather(
                        out_ap=gt, in_ap=table_ap,
                        idxs_ap=idxs_row,
                        num_idxs=2048, num_idxs_reg=2048,
                        elem_size=width)
                else:
                    for h in range(2):
                        nc.gpsimd.dma_gather(
                            out_ap=gt[:, h * 8:(h + 1) * 8, :],
                            in_ap=table_ap,
                            idxs_ap=idxs_row[:, h * 64:(h + 1) * 64],
                            num_idxs=1024, num_idxs_reg=1024,
                            elem_size=width)
                return alloc if alloc is not None else gt

            def attn_scores(pool, kk, qq, awp):
                """Both batches at once.  kk: (128, KNB, 2, H, DK) bf16 AP
                (gathered K rows); qq: (128, 2, H, DK) bf16 AP; awp:
                (128, 2, KNB, H, 2) bf16 AP to receive duplicated
                softmax weights (contiguous per batch)."""
                q5 = qq.unsqueeze(1).broadcast_to((128, KNB, 2, H, DK))
                tmp = pool.tile([128, KNB, 2, H, DK], BF, tag="sc_tmp",
                                bufs=1)
                nc.vector.tensor_mul(tmp[:], kk, q5)
                # reduce over d=32 via add tree (bf16, final step fp32)
                t1 = pool.tile([128, KNB, 2, H, 16], BF, tag="sc_t1", bufs=1)
                nc.vector.tensor_add(t1[:], tmp[:, :, :, :, 0:16],
                                     tmp[:, :, :, :, 16:32])
                t2 = pool.tile([128, KNB, 2, H, 8], BF, tag="sc_t2", bufs=1)
                nc.vector.tensor_add(t2[:], t1[:, :, :, :, 0:8],
                                     t1[:, :, :, :, 8:16])
                t3 = pool.tile([128, KNB, 2, H, 4], BF, tag="sc_t3", bufs=1)
                nc.vector.tensor_add(t3[:], t2[:, :, :, :, 0:4],
                                     t2[:, :, :, :, 4:8])
                t4 = pool.tile([128, KNB, 2, H, 2], BF, tag="sc_t4", bufs=1)
                nc.vector.tensor_add(t4[:], t3[:, :, :, :, 0:2],
                                     t3[:, :, :, :, 2:4])
                scores = pool.tile([128, KNB, 2, H], F32, tag="sc_sc")
                nc.vector.tensor_add(scores[:], t4[:, :, :, :, 0],
                                     t4[:, :, :, :, 1])
                pexp = pool.tile([128, KNB, 2, H], F32, tag="sc_pe")
                nc.scalar.activation(pexp[:], scores[:], Exp, scale=ISQ)
                rsum = pool.tile([128, 2, H], F32, tag="sc_rs")
                nc.vector.reduce_sum(
                    rsum[:], pexp[:].rearrange("p j b h -> p b h j"), axis=X)
                rrec = pool.tile([128, 2, H], F32, tag="sc_rr")
                nc.vector.reciprocal(rrec[:], rsum[:])
                for b in range(2):
                    rrb = rrec[:, b].unsqueeze(1) \
                        .broadcast_to((128, KNB, H))
                    nc.vector.tensor_mul(awp[:, b, :, :, 0],
                                         pexp[:, :, b, :], rrb)
                    nc.vector.tensor_mul(awp[:, b, :, :, 1],
                                         pexp[:, :, b, :], rrb)

            def attn_values(pool, xx0, xx1, awp, res2):
                """xx0/xx1: (128, KNB, 256) bf16 value rows per batch; awp:
                (128, 2, KNB, H, 2); res2: (128, 2, H, DK) fp32 AP."""
                tmp2 = pool.tile([128, KNB, 2, H, DK], BF, tag="va_tmp",
                                 bufs=1)
                for b, xx in ((0, xx0), (1, xx1)):
                    x5 = xx.rearrange("p j (h x y) -> p j h x y", y=2, x=16)
                    o5 = tmp2[:, :, b].rearrange(
                        "p j h (x y) -> p j h x y", y=2)
                    aw5 = awp[:, b].unsqueeze(3) \
                        .broadcast_to((128, KNB, H, DK // 2, 2))
                    nc.vector.tensor_mul(o5, x5, aw5)
                # reduce over j=16 via add tree
                v1 = pool.tile([128, 8, 2, H, DK], BF, tag="va_v1", bufs=1)
                nc.vector.tensor_add(v1[:], tmp2[:, 0:8], tmp2[:, 8:16])
                v2 = pool.tile([128, 4, 2, H, DK], BF, tag="va_v2", bufs=1)
                nc.vector.tensor_add(v2[:], v1[:, 0:4], v1[:, 4:8])
                v3 = pool.tile([128, 2, 2, H, DK], BF, tag="va_v3", bufs=1)
                nc.vector.tensor_add(v3[:], v2[:, 0:2], v2[:, 2:4])
                nc.vector.tensor_add(res2, v3[:, 0], v3[:, 1])

            # ------------- phase 3: edge attention ---------------------
            with (
                tc.tile_pool(name="eat", bufs=2) as eat,
                tc.tile_pool(name="eatw", bufs=2) as eatw,
            ):
                for mt in range(MT):
                    g = gather_tile(eat, "kg_e", ag2_out[:],
                                    gie_sb[:, mt, :], TW)
                    awp = eatw.tile([128, 2, KNB, H, 2], BF, tag="e_awp")
                    attn_scores(
                        eatw,
                        g[:, :, 0:2 * DV].rearrange(
                            "p j (b h d) -> p j b h d", b=2, d=DK),
                        eq[:, :, mt, :].rearrange(
                            "p b (h d) -> p b h d", d=DK),
                        awp[:])
                    res2 = eatw.tile([128, 2, H, DK], F32, tag="eres")
                    attn_values(eatw,
                                g[:, :, 2 * DV:3 * DV],
                                g[:, :, 3 * DV:4 * DV],
                                awp[:], res2[:])
                    nc.scalar.copy(
                        ae[:, mt, :, :],
                        res2[:].rearrange("p b h d -> p b (h d)"))

                if stage < 3:
                    for b in range(B):
                        for vt in range(VT):
                            dbg3 = eatw.tile([128, DV], F32, tag="dbg3")
                            nc.scalar.copy(dbg3[:], ae[:, vt, b, :])
                            nc.sync.dma_start(
                                out=out[b, vt * 128:(vt + 1) * 128, :],
                                in_=dbg3[:])
                    return nc

            with tc.tile_pool(name="vqg", bufs=1) as vqgp:
                vqgt = vqgp.tile([128, VT, KNB, 2 * DV], BF, tag="vqg")
                awpv = vqgp.tile([128, VT, 2, KNB, H, 2], BF, tag="awpv")

                # ------------- phase 4: x_vert partial = d0n^T @ ae ----
                with (
                    tc.tile_pool(name="s6p", bufs=4) as s6p,
                    tc.tile_pool(name="ps6", bufs=2, space="PSUM") as ps6p,
                ):
                    # prefetch v_Q gathers + vertex softmax weights: the
                    # gathers' SWDGE desc-gen and the score DVE work all
                    # hide behind the phase-4 matmuls / AllReduce.
                    for vt in range(VT):
                        gather_tile(vqgt[:, vt], "vqg", ag1_out[:],
                                    giv_sb[:, vt, :], 2 * DV)
                        attn_scores(
                            s6p,
                            vqgt[:, vt].rearrange(
                                "p j (b h d) -> p j b h d", b=2, d=DK),
                            vk[:, :, vt, :].rearrange(
                                "p b (h d) -> p b h d", d=DK),
                            awpv[:, vt])
                    for ck in range(NT // 8):
                        for b in range(B):
                            dts = []
                            for mt in range(MT):
                                dn_t = s6p.tile([128, 8 * 128], BF,
                                                tag="d0n", bufs=16)
                                nc.sync.dma_start(
                                    out=dn_t[:],
                                    in_=d0n[b, mt * 128:(mt + 1) * 128,
                                            ck * 1024:(ck + 1) * 1024])
                                dts.append(dn_t)
                            # two 4-bank accumulation groups (one group per
                            # PSUM generation; start= must own its banks)
                            for gr in range(2):
                                pss = [ps6p.tile([128, DV], F32,
                                                 tag=f"s6_{i}",
                                                 name=f"s6_{gr}_{i}")
                                       for i in range(4)]
                                for mt in range(MT):
                                    for i4 in range(4):
                                        i = gr * 4 + i4
                                        nc.tensor.matmul(
                                            pss[i4][:],
                                            lhsT=dts[mt][:,
                                                         i * 128:(i + 1) * 128],
                                            rhs=ae[:, mt, b, :],
                                            start=(mt == 0),
                                            stop=(mt == MT - 1))
                                for i4 in range(4):
                                    i = gr * 4 + i4
                                    xv_t = s6p.tile([128, DV], BF, tag="xv_t")
                                    nc.scalar.copy(xv_t[:], pss[i4][:])
                                    r0 = (ck * 8 + i) * 128
                                    nc.sync.dma_start(
                                        out=ar_in[r0:r0 + 128,
                                                  b * DV:(b + 1) * DV],
                                        in_=xv_t[:])
                        if ck == 1:
                            # AR half A: rows 0..N/2 done; overlaps the
                            # second half of phase 4
                            nc.gpsimd.collective_compute(
                                "AllReduce", AluOpType.add, replica_groups=rg,
                                ins=[ar_in[0:N // 2, :].opt()],
                                outs=[ara_out[:].opt()])
                            nc.sync.dma_start(out=arl[0:N // 2, :],
                                              in_=ara_out[:])
                    nc.gpsimd.collective_compute(
                        "AllReduce", AluOpType.add, replica_groups=rg,
                        ins=[ar_in[N // 2:N, :].opt()],
                        outs=[arb_out[:].opt()])
                    nc.sync.dma_start(out=arl[N // 2:N, :], in_=arb_out[:])

                if stage < 5:
                    # out[0] = v_Q[v_idx[n,0]] (vqgt path),
                    # out[1] = x_vert[b0][v_idx[n,0]] (ar path)
                    with tc.tile_pool(name="vat0", bufs=2) as vat0:
                        for vt in range(VT):
                            g = gather_tile(vat0, "dbg4", arl[:],
                                            giv_sb[:, vt, :], 2 * DV)
                            dbg4q = vat0.tile([128, DV], F32, tag="dbg4q")
                            nc.scalar.copy(dbg4q[:], vqgt[:, vt, 0, 0:DV])
                            nc.sync.dma_start(
                                out=out[0, vt * 128:(vt + 1) * 128, :],
                                in_=dbg4q[:])
                            dbg4f = vat0.tile([128, DV], F32, tag="dbg4f")
                            nc.scalar.copy(dbg4f[:], g[:, 0, 0:DV])
                            nc.sync.dma_start(
                                out=out[1, vt * 128:(vt + 1) * 128, :],
                                in_=dbg4f[:])
                    return nc

                # ------------- phase 5: vertex attention (values only) --
                with (
                    tc.tile_pool(name="vat", bufs=2) as vat,
                    tc.tile_pool(name="vatw", bufs=2) as vatw,
                ):
                    for vt in range(VT):
                        xg = gather_tile(vat, "xg_v", arl[:],
                                         giv_sb[:, vt, :], 2 * DV)
                        res2 = vatw.tile([128, 2, H, DK], F32, tag="vres")
                        attn_values(vatw,
                                    xg[:, :, 0:DV],
                                    xg[:, :, DV:2 * DV],
                                    awpv[:, vt], res2[:])
                        for b in range(B):
                            nc.sync.dma_start(
                                out=out[b, vt * 128:(vt + 1) * 128, :],
                                in_=res2[:, b].rearrange("p h d -> p (h d)"))

    return nc


def _pack_idx(L):
    """L: (T, n) int array of table-row indices (j-major per tile) ->
    (128, T*(n//16)) int16 dma_gather index layout (16-part wrap, 8x repl)."""
    T, n = L.shape
    a = L.reshape(T, n // 16, 16).transpose(2, 0, 1).reshape(16, T * (n // 16))
    return np.tile(a, (8, 1)).astype(np.int16)


def _prep_core_inputs(c, x_v, x_e, d_0, w5_bf, v_idx, e_idx, xvt_full):
    sh_e = slice(c * MS, (c + 1) * MS)
    sh_v = slice(c * NS, (c + 1) * NS)

    d0s = d_0[:, sh_e, :]
    d0n_c = np.ascontiguousarray(d0s).astype(BF16)
    d0t_c = np.ascontiguousarray(d0s.transpose(0, 2, 1)).astype(BF16)
    xvt_s = np.ascontiguousarray(
        x_v[:, sh_v, :].transpose(0, 2, 1)).astype(BF16)
    xet_s = np.ascontiguousarray(
        x_e[:, sh_e, :].transpose(0, 2, 1)).astype(BF16)

    # edge gather rows: AllGather output is rank-major = global edge id
    e = e_idx[sh_e].astype(np.int64)
    Le = e.reshape(MT, 128, KNB).transpose(0, 2, 1).reshape(MT, KNB * 128)
    gie_np = _pack_idx(Le)

    v = v_idx[sh_v].astype(np.int64).reshape(VT, 128, KNB)
    Lv = v.transpose(0, 2, 1).reshape(VT, KNB * 128)
    giv_np = _pack_idx(Lv)

    return {
        "xvt_f": xvt_full,
        "xvt_s": xvt_s,
        "xet_s": xet_s,
        "w5": w5_bf,
        "d0t": d0t_c,
        "d0n": d0n_c,
        "gie": gie_np,
        "giv": giv_np,
    }


def run(inputs, trace=False):
    x_v = np.asarray(inputs["x_v"], np.float32)
    x_e = np.asarray(inputs["x_e"], np.float32)
    d_0 = np.asarray(inputs["d_0"], np.float32)
    v_idx = np.asarray(inputs["v_idx"])
    e_idx = np.asarray(inputs["e_idx"])
    w5_bf = np.ascontiguousarray(np.stack([
        np.asarray(inputs["W_vQ"]).T, np.asarray(inputs["W_vK"]).T,
        np.asarray(inputs["W_vV"]).T, np.asarray(inputs["W_eQ"]).T,
        np.asarray(inputs["W_eK"]).T])).astype(BF16)
    xvt_full = np.ascontiguousarray(x_v.transpose(0, 2, 1)).astype(BF16)

    stage = int(os.environ.get("KSTAGE", "5"))
    if ("nc", stage) not in _CACHE:
        _CACHE[("nc", stage)] = _build_module(stage)
    nc = _CACHE[("nc", stage)]

    in_maps = [
        _prep_core_inputs(c, x_v, x_e, d_0, w5_bf, v_idx, e_idx, xvt_full)
        for c in range(W)
    ]
    try:
        r = run_bass_kernel_spmd(nc, in_maps, core_ids=list(range(W)),
                                 trace=trace)
    except ModuleNotFoundError:
        r = run_bass_kernel_spmd(nc, in_maps, core_ids=list(range(W)),
                                 trace=False)
    outs = [r.results[c]["out"] for c in range(W)]
    full = np.concatenate(outs, axis=1).astype(np.float32)
    return full, r.exec_time_ns


def kernel(**inputs):
    full, _ = run(inputs, trace=False)
    return full
